# revision 38
# baseline (speedup 1.0000x reference)
"""Trainium2 Bass kernel for nn_LstmNetTest (2x LSTM + FC head).

Strategy (8 NeuronCores):
  - The dominant cost is xg1 = x_flat @ Wih1.T  ([256, 346112] x [346112, 64]).
    Shard the contraction dim K=346112 across 8 cores (43264 each); each core
    computes a partial [64, 256] gate projection with 338 accumulating
    matmuls (stationary = Wih1^T K-tile [128,64], moving = x^T K-tile [128,256]).
  - The tiny LSTM2 input projection ([5] -> [64]) is computed on every core
    with weights pre-scaled by 1/8 so the AllReduce sum is exact.
  - Both projections are rearranged into an "alt" gate tensor [32, 4*256]
    (partitions = state dims of both LSTMs, free = gate_type*256 + b*32 + t)
    and AllReduce-summed across cores.
  - Every core then runs the fused 32-step double-LSTM recurrence: per step
    4 tiny matmuls (one per gate type, both LSTMs in one [32,32] stationary
    block-diag weight) + partition-aligned elementwise ops, then the FC head
    (which also folds the h1+h2 sum via a stacked stationary). Output is
    taken from core 0.

Host side only reshapes/transposes/shards (no arithmetic on activations).
"""

import sys

for _p in ("/opt/trn_rl_repo",):
    if _p not in sys.path:
        sys.path.insert(0, _p)

import numpy as np

# Problem constants (hardcoded per contest rules)
B, S = 8, 32
H = 16
D1 = 128 * 52 * 52  # 346112
D2 = 5
G = 4 * H  # 64
M = B * S  # 256
NCORES = 8
KSH = D1 // NCORES  # 43264
KTILES = KSH // 128  # 338
CHUNK = 13  # K-tiles per DMA chunk
NCHUNK = KTILES // CHUNK  # 26

USE_F32R = True  # fp32 matmul at full PE rate (moving free dim 256 >= 256)
USE_BF16_W = False  # walrus rejects mixed f32r x bf16 matmul inputs
USE_BF16_GEMM = True  # DMA W as fp16 and cast x->fp16 on the (idle) DVE

_CACHE = {}

# gate-type order we use: a = 0:i, 1:f, 2:o, 3:g~ ; torch row blocks i,f,g,o
_TAU = [0, 1, 3, 2]


def _gate_perm():
    # rows of Wih* reordered to our (a, p) order
    return np.concatenate([np.arange(t * H, (t + 1) * H) for t in _TAU])


def _build_bass(num_devices=NCORES, phases="full"):
    import concourse.bacc as bacc
    import concourse.mybir as mybir
    import concourse.tile as tile

    F32 = mybir.dt.float32
    FIN = mybir.dt.float32r if USE_F32R else F32
    WDT = mybir.dt.float16 if USE_BF16_GEMM else (mybir.dt.bfloat16 if USE_BF16_W else FIN)
    BF16 = mybir.dt.float16
    if USE_BF16_GEMM:
        FIN = F32  # x arrives fp32, cast to bf16 on-device before the matmul
    ACT = mybir.ActivationFunctionType

    nc = bacc.Bacc(
        "TRN2",
        target_bir_lowering=False,
        debug=False,
        num_devices=num_devices,
    )

    xs_d = nc.dram_tensor("xs", [NCHUNK, 128, CHUNK * M], FIN, kind="ExternalInput")
    ws_d = nc.dram_tensor("ws", [NCHUNK, 128, CHUNK * G], WDT, kind="ExternalInput")
    lt_d = nc.dram_tensor("lt", [D2, M], F32, kind="ExternalInput")
    w2t_d = nc.dram_tensor("w2t", [D2, G], F32, kind="ExternalInput")
    # whh: [32, 4*32] — per gate type a, block-diag [k=(l',p'), m=(l,p)]
    whh_d = nc.dram_tensor("whh", [2 * H, 4 * 2 * H], F32, kind="ExternalInput")
    b1c_d = nc.dram_tensor("b1c", [G, 1], F32, kind="ExternalInput")
    b2c_d = nc.dram_tensor("b2c", [G, 1], F32, kind="ExternalInput")
    fcwt_d = nc.dram_tensor("fcwt", [2 * H, 4], F32, kind="ExternalInput")
    fcb_d = nc.dram_tensor("fcb", [4, 1], F32, kind="ExternalInput")
    eye_d = nc.dram_tensor("eye", [2 * H, 2 * H], F32, kind="ExternalInput")
    out_d = nc.dram_tensor("out", [4, M], F32, kind="ExternalOutput")

    with tile.TileContext(nc) as tc:
        with (
            tc.tile_pool(name="xp", bufs=3) as xp,
            tc.tile_pool(name="wp", bufs=3) as wp,
            tc.tile_pool(name="const", bufs=1) as cp,
            tc.tile_pool(name="state", bufs=1) as stp,
            tc.tile_pool(name="step", bufs=4) as spp,
            tc.tile_pool(name="acc", bufs=1, space="PSUM") as psp,
            tc.tile_pool(name="psg", bufs=2, space="PSUM") as psg,
            tc.tile_pool(name="dram", bufs=1, space="DRAM") as dp,
        ):
            psum1 = psp.tile([G, M], F32)
            psum2 = psp.tile([G, M], F32)

            # Big GEMM: partial xg1 = Wih1_shard @ x_shard^T  -> [64, 256]
            # (bulk DMAs own the SP ring)
            n_mm = NCHUNK * CHUNK
            for c in range(NCHUNK):
                x_t = xp.tile([128, CHUNK * M], FIN)
                w_t = wp.tile([128, CHUNK * G], WDT)
                nc.sync.dma_start(x_t[:], xs_d[c])
                nc.sync.dma_start(w_t[:], ws_d[c])
                if USE_BF16_GEMM:
                    xb_t = xp.tile([128, CHUNK * M], BF16, tag="xb")
                    nc.vector.tensor_copy(xb_t[:], x_t[:])
                    mm_x = xb_t
                else:
                    mm_x = x_t
                njs = 1 if phases == "dma" else CHUNK
                for j in range(njs):
                    idx = c * CHUNK + j
                    nc.tensor.matmul(
                        psum1[:],
                        w_t[:, j * G : (j + 1) * G],
                        mm_x[:, j * M : (j + 1) * M],
                        start=(c == 0 and j == 0),
                        stop=(idx == n_mm - 1 or (phases == "dma" and c == NCHUNK - 1)),
                    )

            # constants (issued after the bulk stream)
            lt_t = cp.tile([D2, M], F32)
            nc.sync.dma_start(lt_t[:], lt_d[:])
            w2t_t = cp.tile([D2, G], F32)
            nc.sync.dma_start(w2t_t[:], w2t_d[:])
            whh_t = cp.tile([2 * H, 4 * 2 * H], F32)
            nc.sync.dma_start(whh_t[:], whh_d[:])
            b1c_t = cp.tile([G, 1], F32)
            nc.sync.dma_start(b1c_t[:], b1c_d[:])
            b2c_t = cp.tile([G, 1], F32)
            nc.sync.dma_start(b2c_t[:], b2c_d[:])
            fcwt_t = cp.tile([2 * H, 4], F32)
            nc.sync.dma_start(fcwt_t[:], fcwt_d[:])
            fcb_t = cp.tile([4, 1], F32)
            nc.sync.dma_start(fcb_t[:], fcb_d[:])
            eye_t = cp.tile([2 * H, 2 * H], F32)
            nc.sync.dma_start(eye_t[:], eye_d[:])

            # LSTM2 input projection (weights pre-scaled by 1/NCORES)
            nc.tensor.matmul(psum2[:], w2t_t[:], lt_t[:], start=True, stop=True)

            # PSUM -> SBUF with the per-core bias share (bias/8) folded in,
            # then DMA into the "alt" DRAM layout [32, 4*256] and AllReduce.
            sb1 = stp.tile([G, M], F32)
            sb2 = stp.tile([G, M], F32)
            nc.vector.tensor_add(sb1[:], psum1[:], b1c_t[:].broadcast_to([G, M]))
            nc.vector.tensor_add(sb2[:], psum2[:], b2c_t[:].broadcast_to([G, M]))
            arin = dp.tile([2 * H, 4 * M], F32)
            arout = dp.tile([2 * H, 4 * M], F32)
            # arin element (l, p, a, n) at [l*16+p, a*256+n]; sb_l stream is
            # (a, p, n)-ordered, so view arin as [l][a, p, n]. The two DMAs go
            # to different HWDGE rings (SP vs ACT) so their ~2us fixed
            # latencies overlap.
            arin_v = arin[:].rearrange("(l p) (a n) -> l a p n", l=2, a=4)
            nc.sync.dma_start(arin_v[0], sb1[:])
            nc.scalar.dma_start(arin_v[1], sb2[:])
            if num_devices > 1:
                nc.gpsimd.collective_compute(
                    "AllReduce",
                    mybir.AluOpType.add,
                    replica_groups=[list(range(num_devices))],
                    ins=[arin[:].opt()],
                    outs=[arout[:].opt()],
                )
            else:
                nc.sync.dma_start(arout[:], arin[:])

            # xg: [32, (a=4, b=8, t=32)] — bias already folded in pre-AR
            xg = stp.tile([2 * H, 4 * M], F32)
            nc.sync.dma_start(xg[:], arout[:])

            if phases != "full":
                nc.sync.dma_start(out_d[:], xg[0:4, 0:M])

            # Fused double-LSTM recurrence.
            # State rows: h1/c1 [0:16], h2/c2 [16:32]. Gates in free dim.
            # The batch is split into NQ independent chains, interleaved per
            # step so one chain's cross-engine handoff latency hides under the
            # other chains' execution.
            if phases == "full":
                NQ = 2
                BB = B // NQ
                hs = stp.tile([2 * H, M], F32)  # free = b*32 + t
                h0 = stp.tile([2 * H, BB], F32)
                nc.any.memset(h0[:], 0.0)
                cts = []
                for q in range(NQ):
                    ctq = stp.tile([2 * H, BB], F32, tag=f"ct{q}")
                    nc.any.memset(ctq[:], 0.0)
                    cts.append(ctq)
                xg_v = xg[:].rearrange("p (a b t) -> p t a b", a=4, t=S)
                hs_v = hs[:].rearrange("p (b t) -> p t b", t=S)
                # sigmoid(x) = 0.5*tanh(x/2) + 0.5: the x/2 for the i/f/o gates
                # is pre-baked into the weights on the host, so each step needs
                # a single un-scaled Tanh over all 4 gates (Sigmoid+Tanh share
                # no ACT table; per-instruction function switches would cost a
                # 1283ns table load). The +xg add is done on the PE via an
                # identity-matmul accumulation so ACT can read PSUM directly.
                for t in range(S):
                    for q in range(NQ):
                        bsl = slice(q * BB, (q + 1) * BB)
                        ct = cts[q]
                        h_prev = h0[:] if t == 0 else hs_v[:, t - 1, bsl]
                        pg = psg.tile([2 * H, 4 * BB], F32, tag=f"pg{q}")
                        nc.tensor.matmul(
                            pg[:].rearrange("p (a b) -> p a b", a=4),
                            eye_t[:],
                            xg_v[:, t, :, bsl],
                            start=True,
                            stop=False,
                            skip_group_check=True,
                        )
                        for a in range(4):
                            nc.tensor.matmul(
                                pg[:, a * BB : (a + 1) * BB],
                                whh_t[:, a * 2 * H : (a + 1) * 2 * H],
                                h_prev,
                                start=False,
                                stop=(a == 3),
                                skip_group_check=True,
                            )
                        g = spp.tile([2 * H, 4 * BB], F32, tag=f"g{q}")
                        nc.scalar.activation(g[:], pg[:], ACT.Tanh)
                        nc.vector.tensor_scalar(
                            g[:, 0 : 3 * BB],
                            g[:, 0 : 3 * BB],
                            0.5,
                            0.5,
                            mybir.AluOpType.mult,
                            mybir.AluOpType.add,
                        )
                        t1 = spp.tile([2 * H, BB], F32, tag=f"t1_{q}")
                        t2 = spp.tile([2 * H, BB], F32, tag=f"t2_{q}")
                        nc.vector.tensor_mul(t1[:], g[:, BB : 2 * BB], ct[:])
                        nc.vector.tensor_mul(
                            t2[:], g[:, 0:BB], g[:, 3 * BB : 4 * BB]
                        )
                        nc.vector.tensor_add(ct[:], t1[:], t2[:])
                        th = spp.tile([2 * H, BB], F32, tag=f"th{q}")
                        nc.scalar.activation(th[:], ct[:], ACT.Tanh)
                        nc.vector.tensor_mul(
                            hs_v[:, t, bsl], g[:, 2 * BB : 3 * BB], th[:]
                        )

                # FC head: out^T [4, 256] = fcW @ h1 + fcW @ h2 + fcb
                pf = psp.tile([4, M], F32)
                nc.tensor.matmul(pf[:], fcwt_t[:], hs[:], start=True, stop=True)
                outt = stp.tile([4, M], F32)
                nc.vector.tensor_add(outt[:], pf[:], fcb_t[:].broadcast_to([4, M]))
                nc.sync.dma_start(out_d[:], outt[:])

    nc.compile()
    return nc


def _prep_inputs(x, l, Wih1, Whh1, bih1, bhh1, Wih2, Whh2, bih2, bhh2, fcW, fcb):
    perm = _gate_perm()
    f32 = np.float32

    xf = np.asarray(x, f32).reshape(M, D1)
    # i/f/o rows (a < 3) carry the extra 1/2 for the sigmoid-via-tanh trick
    hsc = np.repeat([0.5, 0.5, 0.5, 1.0], H)[:, None].astype(f32)  # [64, 1]
    W1p = np.asarray(Wih1, f32)[perm] * hsc  # [64, D1]

    lt = np.ascontiguousarray(np.asarray(l, f32).reshape(M, D2).T)  # [5, 256]
    w2t = np.ascontiguousarray((np.asarray(Wih2, f32)[perm] * hsc / NCORES).T)  # [5, 64]

    # whh [32, 4*32]: per gate type a: block-diag over the two LSTMs,
    # whh[:, a*32:(a+1)*32][k, m] with k = prev-h dim, m = out state dim
    W1h = np.asarray(Whh1, f32)  # [64, 16] torch order
    W2h = np.asarray(Whh2, f32)
    whh = np.zeros((2 * H, 4 * 2 * H), f32)
    for a, tau in enumerate(_TAU):
        gsc = 0.5 if a < 3 else 1.0
        blk = whh[:, a * 2 * H : (a + 1) * 2 * H]
        blk[0:H, 0:H] = W1h[tau * H : (tau + 1) * H].T * gsc  # [k', m]
        blk[H : 2 * H, H : 2 * H] = W2h[tau * H : (tau + 1) * H].T * gsc

    # per-core bias shares in psum row order (a, p), incl. the i/f/o 1/2 scale
    b1c = ((np.asarray(bih1, f32) + np.asarray(bhh1, f32))[perm] * hsc[:, 0] / NCORES)
    b2c = ((np.asarray(bih2, f32) + np.asarray(bhh2, f32))[perm] * hsc[:, 0] / NCORES)

    fcwt = np.concatenate([np.asarray(fcW, f32).T] * 2, axis=0)  # [32, 4]
    fcb_c = np.ascontiguousarray(np.asarray(fcb, f32).reshape(4, 1))

    base = dict(
        lt=lt,
        w2t=w2t,
        whh=np.ascontiguousarray(whh),
        b1c=np.ascontiguousarray(b1c.reshape(G, 1)),
        b2c=np.ascontiguousarray(b2c.reshape(G, 1)),
        fcwt=np.ascontiguousarray(fcwt),
        fcb=fcb_c,
        eye=np.eye(2 * H, dtype=f32),
    )

    in_maps = []
    for ci in range(NCORES):
        k0 = ci * KSH
        # x^T shard, chunk-interleaved: [NCHUNK, 128, CHUNK*M]
        xsh = xf[:, k0 : k0 + KSH].T  # [KSH, 256] (view)
        xs = np.ascontiguousarray(
            xsh.reshape(NCHUNK, CHUNK, 128, M).transpose(0, 2, 1, 3)
        ).reshape(NCHUNK, 128, CHUNK * M)
        wsh = W1p[:, k0 : k0 + KSH].T  # [KSH, 64]
        ws = np.ascontiguousarray(
            wsh.reshape(NCHUNK, CHUNK, 128, G).transpose(0, 2, 1, 3)
        ).reshape(NCHUNK, 128, CHUNK * G)
        if USE_BF16_GEMM:
            ws = ws.astype(np.float16)
        elif USE_BF16_W:
            import ml_dtypes

            ws = ws.astype(ml_dtypes.bfloat16)
        in_maps.append(dict(base, xs=xs, ws=ws))
    return in_maps


def _run(inputs, trace=False, trace_kwargs=None):
    from concourse.bass_utils import run_bass_kernel_spmd

    if "nc" not in _CACHE:
        _CACHE["nc"] = _build_bass()
    nc = _CACHE["nc"]

    in_maps = _prep_inputs(**inputs)
    kw = {}
    if trace:
        kw["trace"] = True
        if trace_kwargs:
            kw["trace_kwargs"] = trace_kwargs
    res = run_bass_kernel_spmd(nc, in_maps, list(range(NCORES)), **kw)
    out_t = res.results[0]["out"]  # [4, 256]
    out = np.ascontiguousarray(out_t.reshape(4, B, S).transpose(1, 2, 0))
    return out, res


def kernel(**inputs) -> np.ndarray:
    out, _ = _run(inputs, trace=False)
    return out
